# revision 57
# baseline (speedup 1.0000x reference)
"""Multi-head attention Trainium2 Bass kernel (v2 — ACT-bound design).

Problem: nn_MultiHeadAttention (B=8, D=256, N=2048, H=4, head_dim=64), fp32.
Sharding: data-parallel over batch — each of the 8 NeuronCores handles one
batch element end to end (no communication).

The kernel is structured so the Scalar (ACT) engine — which must exp() all
H*N^2 = 16.8M scores at 1 elem/cycle/lane — runs continuously from ~5us in
to the end; every other engine's work hides under it (~133us of exp).

Per-core design:
  - Scores run in fp8e4m3 with DoubleRow perf mode (2 fp8 MACs/cell/cycle):
    q/k are stored as [128 = 4 heads x 32 d-low, 2 d-high, N] so each head's
    32-partition band holds d = 32*c + dl pairs; one DR matmul contracts all
    64 head dims. fp8 score noise (~0.5% on softmax weights) is well inside
    the error budget; fp8 on the VALUE path would not be (norm-relative
    errors pass through the output projection undiminished), so e/v stay f16.
  - exp(s/8) on ACT reads each PSUM score tile [128m, 1024n] and writes an
    f16 e-tile; e in [0.5, 1.9] so f16 costs ~5e-4 relative.
  - PV: stationary v16[mc, h] = [128 m, 64 v-cols + 64 ones-cols]; the ones
    columns replicate the softmax denominator onto partitions 64..127 of the
    PSUM x-accumulator, so normalization is reciprocal_approx_fast on rows
    64:128 + an elementwise multiply against rows 0:64 — no cross-partition
    broadcast (the old DRAM bounce) needed.
  - Loop nest: window-pair (1024 cols) outer, head inner, m-chunk innermost.
    PV matmuls trail scores by a 4-chunk lag queue and projection/output
    matmuls are drained from a filler queue budgeted per exp-slot, so the PE
    never makes ACT wait.  Output projection for window-pair 0 runs during
    window-pair 1's attention; only wp1's out-proj (~3us) is tail.
  - PSUM: scores [128,1024]x2 (4 banks) + x-accum [128,1024] (2) + shared
    proj/out-proj [128,512]x2 (2) = exactly 8 banks.
"""

from collections import deque

import numpy as np

import concourse.bass as bass
import concourse.bacc as bacc
import concourse.mybir as mybir
import concourse.tile as tile
from concourse.bass_utils import run_bass_kernel_spmd

F32 = mybir.dt.float32
F32R = mybir.dt.float32r
F16 = mybir.dt.float16
FP8 = mybir.dt.float8e4
EXP = mybir.ActivationFunctionType.Exp
DR = mybir.MatmulPerfMode.DoubleRow

B, D, N, H = 8, 256, 2048, 4
HD = D // H   # 64
P = 128
DC = D // P   # 2 d-chunks (contraction for projections)
MC = N // P   # 16 m-chunks
NW = 512      # projection / matmul free-dim chunk
WIN = 1024    # exp window (= score tile width, x-accum width)
PV_LAG = 2    # m-chunks by which PV trails exp


def build_nc(reps: int = 1) -> bass.Bass:
    nc = bacc.Bacc()

    xq_d = nc.declare_dram_parameter("query", [D, N], F32, isOutput=False)
    xk_d = nc.declare_dram_parameter("key", [D, N], F32, isOutput=False)
    xv_d = nc.declare_dram_parameter("value", [D, N], F32, isOutput=False)
    wq_d = nc.declare_dram_parameter("wq", [D, D], F32, isOutput=False)
    wk_d = nc.declare_dram_parameter("wk", [D, D], F32, isOutput=False)
    wv_d = nc.declare_dram_parameter("wv", [D, D], F32, isOutput=False)
    wm_d = nc.declare_dram_parameter("wm", [D, D], F32, isOutput=False)
    bq_d = nc.declare_dram_parameter("bq", [D], F32, isOutput=False)
    bk_d = nc.declare_dram_parameter("bk", [D], F32, isOutput=False)
    bv_d = nc.declare_dram_parameter("bv", [D], F32, isOutput=False)
    bm_d = nc.declare_dram_parameter("bm", [D], F32, isOutput=False)
    out_d = nc.declare_dram_parameter("out", [D, N], F32, isOutput=True)

    with tile.TileContext(nc) as tc:
        for _rep in range(reps):
            with (
                tc.tile_pool(name="persist", bufs=1) as pp,
                tc.tile_pool(name="instage", bufs=3) as isp,
                tc.tile_pool(name="exp_pool", bufs=10) as ep,
                tc.tile_pool(name="rec_pool", bufs=2) as rp,
                tc.tile_pool(name="out_stage", bufs=4) as sp,
                tc.tile_pool(name="psum", bufs=1, space="PSUM") as psp,
                tc.tile_pool(name="dram_scr", bufs=3, space="DRAM") as dsp,
            ):
                # ---- persistent tiles -----------------------------------
                q8 = pp.tile([P, 2, N], FP8, name="q8")
                k8 = pp.tile([P, 2, N], FP8, name="k8")
                # head 3 lives at partitions 96-127, which the PE cannot use
                # as an operand base (must be 0/32/64) — DMA-shift its band
                # into base-0 aux tiles
                q8b = pp.tile([32, 2, N], FP8, name="q8b")
                k8b = pp.tile([32, 2, N], FP8, name="k8b")
                v16 = pp.tile([P, MC, H, 2 * HD], F16, name="v16")
                xst = pp.tile([HD, H, N], F32R, name="xst")
                xq16 = pp.tile([P, DC, N], F16, name="xq16")
                xk16 = pp.tile([P, DC, N], F16, name="xk16")
                xv16 = pp.tile([P, DC, N], F16, name="xv16")
                wq16 = pp.tile([P, DC, D], F16, name="wq16")
                wk16 = pp.tile([P, DC, D], F16, name="wk16")
                wv16 = pp.tile([P, DC, D], F16, name="wv16")
                wm_r = pp.tile([HD, H, D], F32R, name="wm_r")
                bq_sb = pp.tile([P, 2], F32, name="bq_sb")
                bk_sb = pp.tile([P, 2], F32, name="bk_sb")
                bm_sb = pp.tile([P, DC], F32, name="bm_sb")
                bv_bc = pp.tile([P, D], F32, name="bv_bc")

                # warm the exp activation table while DMAs stream
                warm = pp.tile([1, 2], F32)
                nc.vector.memset(warm[:], 0.0)
                nc.scalar.activation(warm[:], warm[:], EXP, scale=0.125)

                # PE p-state ramp: ~4us of back-to-back dummy matmuls during
                # the input DMA wait so the first real projections run at full
                # clock instead of 0.65 GHz
                ramp_w = pp.tile([P, 16], F16, name="ramp_w")
                ramp_mv = pp.tile([P, NW], F16, name="ramp_mv")
                nc.vector.memset(ramp_w[:], 0.0)
                nc.vector.memset(ramp_mv[:], 0.0)
                ramp_ps = psp.tile([P, NW], F32, tag="pj", bufs=2, name="ps_pj")
                for _ in range(12):
                    nc.tensor.matmul(
                        ramp_ps[0:16, :], ramp_w[:], ramp_mv[:], start=True, stop=True
                    )

                # ---- input DMA --------------------------------------------
                # Three parallel DMA queues: SP-HWDGE, ACT-HWDGE, Pool-SWDGE.
                # The Pool (gpsimd) queue can CAST f32->f16 in flight, so the
                # bulk of the inputs loads with no staging/rounding step at
                # all; only the three head-critical chunks (xk nw0, xq nw0/1)
                # go as f32 on the two HW queues in parallel, rounded by the
                # otherwise-idle DVE.
                def stage_x(dram, dst, nw, tag, dma, cp):
                    sl = slice(nw * NW, (nw + 1) * NW)
                    st = isp.tile([P, DC, NW], F32, tag=tag, name=f"st_{tag}")
                    dma.dma_start(st[:], dram.rearrange("(dc p) n -> p dc n", p=P)[:, :, sl])
                    cp.tensor_copy(dst[:, :, sl], st[:])

                def cast_x(dram, dst, nw):
                    sl = slice(nw * NW, (nw + 1) * NW)
                    nc.gpsimd.dma_start(
                        dst[:, :, sl], dram.rearrange("(dc p) n -> p dc n", p=P)[:, :, sl]
                    )

                # weights cast f32->f16 in flight on the Pool queue (no
                # staging); the three head-critical x chunks staged f32 on the
                # SP/ACT queues in parallel and rounded by the idle DVE
                # only the transfers the first scores need go out before the
                # early projections — the shared DMA engine drains descriptors
                # in rough FIFO order, so anything issued here delays them
                nc.gpsimd.dma_start(wk16[:], wk_d.rearrange("(dc p) o -> p dc o", p=P))
                cast_x(xk_d, xk16, 0)
                nc.gpsimd.dma_start(wq16[:], wq_d.rearrange("(dc p) o -> p dc o", p=P))
                stage_x(xq_d, xq16, 0, "xq", nc.scalar, nc.vector)
                stage_x(xq_d, xq16, 1, "xq2", nc.scalar, nc.vector)
                nc.sync.dma_start(bq_sb[:], bq_d.rearrange("(c p) -> p c", p=P))
                nc.sync.dma_start(bk_sb[:], bk_d.rearrange("(c p) -> p c", p=P))
                nc.gpsimd.dma_start(wv16[:], wv_d.rearrange("(dc p) o -> p dc o", p=P))

                # ---- emission helpers -----------------------------------
                def emit_qkproj(w16, x16, b_sb, dst8, c, nw, aux=None):
                    # projection chunk c (fp8 pair dim), n-window nw -> dst8[:, c, nw]
                    sl = slice(nw * NW, (nw + 1) * NW)
                    ps = psp.tile([P, NW], F32, tag="pj", bufs=2, name="ps_pj")
                    for dc in range(DC):
                        nc.tensor.matmul(
                            ps[:],
                            w16[:, dc, c * P : (c + 1) * P],
                            x16[:, dc, sl],
                            start=(dc == 0),
                            stop=(dc == DC - 1),
                        )
                    nc.vector.tensor_add(
                        out=dst8[:, c, sl],
                        in0=ps[:],
                        in1=b_sb[:, c : c + 1].to_broadcast((P, NW)),
                    )
                    if aux is not None:
                        nc.sync.dma_start(aux[:, c, sl], dst8[96:128, c, sl])

                def emit_vproj(mc):
                    ps = psp.tile([P, NW], F32, tag="pj", bufs=2, name="ps_pj")
                    for dc in range(DC):
                        nc.tensor.matmul(
                            ps[:, 0:D],
                            xv16[:, dc, mc * P : (mc + 1) * P],
                            wv16[:, dc, :],
                            start=(dc == 0),
                            stop=(dc == DC - 1),
                        )
                    nc.vector.tensor_add(
                        out=v16[:, mc, :, 0:HD],
                        in0=ps[:, 0:D].rearrange("p (h e) -> p h e", e=HD),
                        in1=bv_bc[:].rearrange("p (h e) -> p h e", e=HD),
                    )

                def oproj_mm(out_ap, wp, oc, nw, h, start, stop):
                    n0 = wp * WIN + nw * NW
                    nc.tensor.matmul(
                        out_ap,
                        wm_r[:, h, oc * P : (oc + 1) * P],
                        xst[:, h, n0 : n0 + NW],
                        start=start,
                        stop=stop,
                    )

                def oproj_store(in_ap, wp, oc, nw, dma=None, act_add=False):
                    # bias-add reads PSUM: DVE mid-kernel, ACT (idle once the
                    # exps are done) for the tail stores
                    n0 = wp * WIN + nw * NW
                    o_sb = sp.tile([P, NW], F32, tag="ost", name="o_sb")
                    if act_add:
                        nc.scalar.add(o_sb[:], in_ap, bm_sb[:, oc : oc + 1])
                    else:
                        nc.vector.tensor_add(
                            out=o_sb[:],
                            in0=in_ap,
                            in1=bm_sb[:, oc : oc + 1].to_broadcast((P, NW)),
                        )
                    (dma or nc.sync).dma_start(
                        out_d.rearrange("(c p) n -> p c n", p=P)[:, oc, n0 : n0 + NW],
                        o_sb[:],
                    )

                def emit_oproj(wp, oc, nw, dma=None):
                    # out chunk [128 o, 512 n]; accumulate over heads
                    ps = psp.tile([P, NW], F32, tag="pj", bufs=2, name="ps_pj")
                    for h in range(H):
                        oproj_mm(ps[:], wp, oc, nw, h, h == 0, h == H - 1)
                    oproj_store(ps[:], wp, oc, nw, dma=dma)

                def push_oproj_fillers(wp, oc, nw):
                    # per-head matmul fillers so pops stay fine-grained
                    cell = {}

                    def start(oc=oc, nw=nw):
                        cell["ps"] = psp.tile([P, NW], F32, tag="pj", bufs=2, name="ps_pj")
                        oproj_mm(cell["ps"][:], wp, oc, nw, 0, True, False)

                    filler.append((512, start))
                    for hh in range(1, H - 1):
                        filler.append((512, lambda hh=hh: oproj_mm(cell["ps"][:], wp, oc, nw, hh, False, False)))

                    def fin(oc=oc, nw=nw):
                        oproj_mm(cell["ps"][:], wp, oc, nw, H - 1, False, True)
                        oproj_store(cell["ps"][:], wp, oc, nw)

                    filler.append((512, fin))

                # ---- fillers: (estimated PE cycles, closure) ------------
                # deadline order: PV(h0, mc) fires at exp-slot mc+PV_LAG and
                # scores(h0, mc) at slot mc, so vproj(mc) and kproj(nw) must
                # pop (at ~2/slot in the first block) before those slots
                filler = deque()

                def fv(mc):
                    filler.append((512, lambda: emit_vproj(mc)))

                def fk(c, nw):
                    filler.append((1024, lambda: emit_qkproj(wk16, xk16, bk_sb, k8, c, nw, aux=k8b)))

                fv(0); fv(1); fk(0, 1); fk(1, 1); fv(2); fv(3); fv(4); fv(5)
                fk(0, 2); fk(1, 2); fv(6); fv(7); fv(8); fv(9); fk(0, 3); fk(1, 3)
                for mc in range(10, 16):
                    fv(mc)
                for nw in range(2, 4):
                    for c in range(2):
                        filler.append((1024, lambda c=c, nw=nw: emit_qkproj(wq16, xq16, bq_sb, q8, c, nw, aux=q8b)))

                def pop_fillers(budget):
                    while filler and budget > 0:
                        cyc, fn = filler.popleft()
                        fn()
                        budget -= cyc

                # ---- early projections (feed wp0/h0 scores) -------------
                for c in range(2):
                    emit_qkproj(wk16, xk16, bk_sb, k8, c, 0, aux=k8b)
                for nw in range(2):
                    for c in range(2):
                        emit_qkproj(wq16, xq16, bq_sb, q8, c, nw, aux=q8b)

                # bulk loads behind the head-critical transfers: biases, the
                # ones columns of every PV stationary (denominator rows),
                # remaining x chunks, and the f32r out-proj weight
                nc.sync.dma_start(
                    bv_bc[:], bv_d[:].rearrange("(a o) -> a o", a=1).to_broadcast((P, D))
                )
                nc.sync.dma_start(bm_sb[:], bm_d.rearrange("(c p) -> p c", p=P))
                cast_x(xv_d, xv16, 0)
                cast_x(xk_d, xk16, 1)
                nc.gpsimd.memset(v16[:, :, :, HD : 2 * HD], 1.0)
                cast_x(xv_d, xv16, 1)
                cast_x(xk_d, xk16, 2)
                cast_x(xv_d, xv16, 2)
                cast_x(xk_d, xk16, 3)
                cast_x(xv_d, xv16, 3)
                cast_x(xq_d, xq16, 2)
                cast_x(xq_d, xq16, 3)
                # wm stays f32r (output-projection precision) — stage + round
                st_wm = isp.tile([HD, H, D], F32, tag="wm", name="st_wm")
                nc.sync.dma_start(st_wm[:], wm_d.rearrange("(h p) o -> p h o", p=HD))
                nc.gpsimd.tensor_copy(wm_r[:], st_wm[:])

                # ---- attention ------------------------------------------
                # PV work queue, carried across blocks: ("pv", fn) entries are
                # rate-limited; ("epi", fn) entries (a block's epilogue, which
                # must be EMITTED after that block's last PV so the dependency
                # tracker orders it correctly) pop for free right after.
                pv_pending = deque()

                def pop_pv(max_pops, lag):
                    pops = 0
                    while pv_pending and pops < max_pops:
                        n_pv = sum(1 for k, _ in pv_pending if k == "pv")
                        if n_pv <= lag and pv_pending[0][0] == "pv":
                            break
                        kind, fn = pv_pending.popleft()
                        fn()
                        if kind == "pv":
                            pops += 1

                def emit_epilogue(h, wp, x_ps, halves=1):
                    # Engines cannot shift data across partitions, so the
                    # denominator (rows 64:128 of the accumulator) reaches
                    # partitions 0:63 via a DRAM bounce, as a 64-partition
                    # broadcast. One DVE copy first frees the single-buffered
                    # PSUM x within the PV pop pause.
                    xu = rp.tile([P, WIN], F32, tag="xu", name="xu")
                    nc.vector.tensor_copy(xu[:], x_ps[:])
                    den_dr = dsp.tile([1, WIN], F32, tag="den", name="den_dr")
                    nc.gpsimd.dma_start(den_dr[:], xu[HD : HD + 1, :])
                    bc = rp.tile([HD, WIN], F32, tag="bc", name="bc")
                    nc.gpsimd.dma_start(bc[:], den_dr[:].to_broadcast((HD, WIN)))
                    hw = WIN // halves
                    for q in range(halves):
                        sl = slice(q * hw, (q + 1) * hw)
                        nc.vector.reciprocal_approx_fast(
                            out=bc[:, sl], in_=bc[:, sl]
                        )
                        nc.vector.tensor_mul(
                            out=xst[:, h, wp * WIN + q * hw : wp * WIN + (q + 1) * hw],
                            in0=xu[0:HD, sl],
                            in1=bc[:, sl],
                        )
                        yield q

                for wp in range(N // WIN):
                    for h in range(H):
                        hb = h * 32
                        last = wp == N // WIN - 1 and h == H - 1
                        o_partials = {}
                        # x accumulator allocated lazily at the first PV pop so
                        # its WAR dep against the previous block's (deferred)
                        # epilogue copy is seen by the dependency tracker
                        cell = {}

                        def get_x(cell=cell):
                            if "x" not in cell:
                                cell["x"] = psp.tile([P, WIN], F32, tag="x", bufs=1, name="x_ps")
                            return cell["x"]

                        def emit_pv(mc, e16, get_x=get_x, h=h):
                            x_ps = get_x()
                            for j in range(WIN // NW):
                                nc.tensor.matmul(
                                    x_ps[:, j * NW : (j + 1) * NW],
                                    v16[:, mc, h, :],
                                    e16[:, j * NW : (j + 1) * NW],
                                    start=(mc == 0),
                                    stop=(mc == MC - 1),
                                )

                        if h < 3:
                            k_src = k8[hb : hb + 32, :, :]
                            q_src = q8[hb : hb + 32, :, :]
                        else:
                            k_src = k8b[:, :, :]
                            q_src = q8b[:, :, :]
                        for mc in range(MC):
                            s_ps = psp.tile([P, WIN], F32, tag="s", bufs=2, name="s_ps")
                            for j in range(WIN // NW):
                                n0 = wp * WIN + j * NW
                                nc.tensor.matmul(
                                    s_ps[:, j * NW : (j + 1) * NW],
                                    k_src[:, :, mc * P : (mc + 1) * P],
                                    q_src[:, :, n0 : n0 + NW],
                                    start=True,
                                    stop=True,
                                    perf_mode=DR,
                                )
                            e16 = ep.tile([P, WIN], F16, tag="e", name="e16")
                            nc.scalar.activation(e16[:], s_ps[:], EXP, scale=0.125)
                            pv_pending.append(
                                ("pv", lambda mc=mc, e16=e16, emit_pv=emit_pv: emit_pv(mc, e16))
                            )
                            pop_fillers(1600 if wp == 0 and h == 0 else 900)
                            # PV pops pause for the first 4 slots of each block
                            # so the previous block's epilogue can free the
                            # single-buffered PSUM accumulator
                            if mc >= 4 or (wp == 0 and h == 0):
                                pop_pv(2, PV_LAG)
                            if last and mc in (6, 9):
                                # pre-accumulate heads 0..2 of the oc=0 output
                                # chunks so only h3's matmul trails the final
                                # epilogue
                                nwp = 0 if mc == 6 else 1
                                ps = psp.tile([P, NW], F32, tag="pj", bufs=2, name="ps_pj")
                                for hh in range(H - 1):
                                    oproj_mm(ps[:], wp, 0, nwp, hh, hh == 0, False)
                                o_partials[(0, nwp)] = ps
                        if last:
                            pop_pv(1 << 30, 0)
                            # tail: copy x out of PSUM per half, then reuse the
                            # freed x banks to pre-accumulate heads 0..2 of the
                            # oc=1 chunks too, so only h3's matmuls + bias +
                            # store trail the per-half epilogue
                            # oc=1 partials go into the score-tile rotation,
                            # free as soon as the second-to-last exp finishes —
                            # they overlap the x copies instead of waiting on
                            # them
                            x2 = psp.tile([P, WIN], F32, tag="s", bufs=2, name="x2")
                            for nw in range(WIN // NW):
                                sl = slice(nw * NW, (nw + 1) * NW)
                                for hh in range(H - 1):
                                    oproj_mm(x2[:, sl], wp, 1, nw, hh, hh == 0, False)
                                o_partials[(1, nw)] = (x2, sl)
                            xx = get_x()
                            dmas = [nc.sync, nc.scalar]
                            for nw in emit_epilogue(h, wp, xx, halves=WIN // NW):
                                ps = o_partials.pop((0, nw))
                                oproj_mm(ps[:], wp, 0, nw, H - 1, False, True)
                                oproj_store(ps[:], wp, 0, nw, dma=dmas[0], act_add=True)
                                x2t, x2sl = o_partials.pop((1, nw))
                                oproj_mm(x2t[:, x2sl], wp, 1, nw, H - 1, False, True)
                                oproj_store(x2t[:, x2sl], wp, 1, nw, dma=dmas[1], act_add=True)
                        else:

                            def epi(h=h, wp=wp, get_x=get_x):
                                for _ in emit_epilogue(h, wp, get_x()):
                                    pass
                                if h == H - 1:
                                    for oc in range(DC):
                                        for nw in range(WIN // NW):
                                            push_oproj_fillers(wp, oc, nw)

                            pv_pending.append(("epi", epi))
                pop_fillers(1 << 30)  # drain anything left

    nc.finalize()
    return nc


_NC_CACHE = None


def _get_nc():
    global _NC_CACHE
    if _NC_CACHE is None:
        _NC_CACHE = build_nc()
    return _NC_CACHE


# fp8 DoubleRow q/k layout: permuted column j = c*128 + h*32 + dl holds
# original output channel o = d*H + h with d = 32*c + dl  (heads in
# 32-partition bands, d split across the fp8 pair dim c).
_QK_PERM = np.empty(D, np.int64)
for _j in range(D):
    _c, _r = divmod(_j, P)
    _h, _dl = divmod(_r, 32)
    _QK_PERM[_j] = (32 * _c + _dl) * H + _h
# v/wm: col j = h*64 + dd maps to o = dd*H + h (head-contiguous)
_V_PERM = np.empty(D, np.int64)
for _j in range(D):
    _h, _dd = divmod(_j, HD)
    _V_PERM[_j] = _dd * H + _h


def make_in_maps(**inputs: np.ndarray) -> list:
    query = np.ascontiguousarray(np.asarray(inputs["query"], np.float32))
    key = np.ascontiguousarray(np.asarray(inputs["key"], np.float32))
    value = np.ascontiguousarray(np.asarray(inputs["value"], np.float32))
    wq = np.ascontiguousarray(np.asarray(inputs["Wq"], np.float32)[:, _QK_PERM])
    wk = np.ascontiguousarray(np.asarray(inputs["Wk"], np.float32)[:, _QK_PERM])
    wv = np.ascontiguousarray(np.asarray(inputs["Wv"], np.float32)[:, _V_PERM])
    wm = np.ascontiguousarray(np.asarray(inputs["Wm"], np.float32)[_V_PERM, :])
    bq = np.ascontiguousarray(np.asarray(inputs["bq"], np.float32)[_QK_PERM])
    bk = np.ascontiguousarray(np.asarray(inputs["bk"], np.float32)[_QK_PERM])
    bv = np.ascontiguousarray(np.asarray(inputs["bv"], np.float32)[_V_PERM])
    bm = np.ascontiguousarray(np.asarray(inputs["bm"], np.float32))

    return [
        {
            "query": query[b],
            "key": key[b],
            "value": value[b],
            "wq": wq,
            "wk": wk,
            "wv": wv,
            "wm": wm,
            "bq": bq,
            "bk": bk,
            "bv": bv,
            "bm": bm,
        }
        for b in range(B)
    ]


def kernel(**inputs: np.ndarray) -> np.ndarray:
    nc = _get_nc()
    in_maps = make_in_maps(**inputs)
    res = run_bass_kernel_spmd(nc, in_maps, core_ids=list(range(B)))
    global _LAST_RESULT
    _LAST_RESULT = res
    return np.stack([r["out"] for r in res.results], axis=0)


_LAST_RESULT = None


# revision 61
# speedup vs baseline: 1.0271x; 1.0271x over previous
"""Multi-head attention Trainium2 Bass kernel (v2 — ACT-bound design).

Problem: nn_MultiHeadAttention (B=8, D=256, N=2048, H=4, head_dim=64), fp32.
Sharding: data-parallel over batch — each of the 8 NeuronCores handles one
batch element end to end (no communication).

The kernel is structured so the Scalar (ACT) engine — which must exp() all
H*N^2 = 16.8M scores at 1 elem/cycle/lane — runs continuously from ~5us in
to the end; every other engine's work hides under it (~133us of exp).

Per-core design:
  - Scores run in fp8e4m3 with DoubleRow perf mode (2 fp8 MACs/cell/cycle):
    q/k are stored as [128 = 4 heads x 32 d-low, 2 d-high, N] so each head's
    32-partition band holds d = 32*c + dl pairs; one DR matmul contracts all
    64 head dims. fp8 score noise (~0.5% on softmax weights) is well inside
    the error budget; fp8 on the VALUE path would not be (norm-relative
    errors pass through the output projection undiminished), so e/v stay f16.
  - exp(s/8) on ACT reads each PSUM score tile [128m, 1024n] and writes an
    f16 e-tile; e in [0.5, 1.9] so f16 costs ~5e-4 relative.
  - PV: stationary v16[mc, h] = [128 m, 64 v-cols + 64 ones-cols]; the ones
    columns replicate the softmax denominator onto partitions 64..127 of the
    PSUM x-accumulator, so normalization is reciprocal_approx_fast on rows
    64:128 + an elementwise multiply against rows 0:64 — no cross-partition
    broadcast (the old DRAM bounce) needed.
  - Loop nest: window-pair (1024 cols) outer, head inner, m-chunk innermost.
    PV matmuls trail scores by a 4-chunk lag queue and projection/output
    matmuls are drained from a filler queue budgeted per exp-slot, so the PE
    never makes ACT wait.  Output projection for window-pair 0 runs during
    window-pair 1's attention; only wp1's out-proj (~3us) is tail.
  - PSUM: scores [128,1024]x2 (4 banks) + x-accum [128,1024] (2) + shared
    proj/out-proj [128,512]x2 (2) = exactly 8 banks.
"""

from collections import deque

import numpy as np

import concourse.bass as bass
import concourse.bacc as bacc
import concourse.mybir as mybir
import concourse.tile as tile
from concourse.bass_utils import run_bass_kernel_spmd

F32 = mybir.dt.float32
F32R = mybir.dt.float32r
F16 = mybir.dt.float16
FP8 = mybir.dt.float8e4
EXP = mybir.ActivationFunctionType.Exp
DR = mybir.MatmulPerfMode.DoubleRow

B, D, N, H = 8, 256, 2048, 4
HD = D // H   # 64
P = 128
DC = D // P   # 2 d-chunks (contraction for projections)
MC = N // P   # 16 m-chunks
NW = 512      # projection / matmul free-dim chunk
WIN = 1024    # exp window (= score tile width, x-accum width)
PV_LAG = 2    # m-chunks by which PV trails exp


def build_nc(reps: int = 1) -> bass.Bass:
    nc = bacc.Bacc()

    xq_d = nc.declare_dram_parameter("query", [D, N], F32, isOutput=False)
    xk_d = nc.declare_dram_parameter("key", [D, N], F32, isOutput=False)
    xv_d = nc.declare_dram_parameter("value", [D, N], F32, isOutput=False)
    wq_d = nc.declare_dram_parameter("wq", [D, D], F32, isOutput=False)
    wk_d = nc.declare_dram_parameter("wk", [D, D], F32, isOutput=False)
    wv_d = nc.declare_dram_parameter("wv", [D, D], F32, isOutput=False)
    wm_d = nc.declare_dram_parameter("wm", [D, D], F32, isOutput=False)
    bq_d = nc.declare_dram_parameter("bq", [D], F32, isOutput=False)
    bk_d = nc.declare_dram_parameter("bk", [D], F32, isOutput=False)
    bv_d = nc.declare_dram_parameter("bv", [D], F32, isOutput=False)
    bm_d = nc.declare_dram_parameter("bm", [D], F32, isOutput=False)
    out_d = nc.declare_dram_parameter("out", [D, N], F32, isOutput=True)

    with tile.TileContext(nc) as tc:
        for _rep in range(reps):
            with (
                tc.tile_pool(name="persist", bufs=1) as pp,
                tc.tile_pool(name="instage", bufs=3) as isp,
                tc.tile_pool(name="exp_pool", bufs=12) as ep,
                tc.tile_pool(name="rec_pool", bufs=2) as rp,
                tc.tile_pool(name="out_stage", bufs=4) as sp,
                tc.tile_pool(name="psum", bufs=1, space="PSUM") as psp,
                tc.tile_pool(name="dram_scr", bufs=3, space="DRAM") as dsp,
            ):
                # ---- persistent tiles -----------------------------------
                q8 = pp.tile([P, 2, N], FP8, name="q8")
                k8 = pp.tile([P, 2, N], FP8, name="k8")
                # head 3 lives at partitions 96-127, which the PE cannot use
                # as an operand base (must be 0/32/64) — DMA-shift its band
                # into base-0 aux tiles
                q8b = pp.tile([32, 2, N], FP8, name="q8b")
                k8b = pp.tile([32, 2, N], FP8, name="k8b")
                v16 = pp.tile([P, MC, H, 2 * HD], F16, name="v16")
                xst = pp.tile([HD, H, N], F32R, name="xst")
                xq16 = pp.tile([P, DC, N], F16, name="xq16")
                xk16 = pp.tile([P, DC, N], F16, name="xk16")
                xv16 = pp.tile([P, DC, N], F16, name="xv16")
                wq16 = pp.tile([P, DC, D], F16, name="wq16")
                wk16 = pp.tile([P, DC, D], F16, name="wk16")
                wv16 = pp.tile([P, DC, D], F16, name="wv16")
                wm_r = pp.tile([HD, H, D], F32R, name="wm_r")
                bq_sb = pp.tile([P, 2], F32, name="bq_sb")
                bk_sb = pp.tile([P, 2], F32, name="bk_sb")
                bm_sb = pp.tile([P, DC], F32, name="bm_sb")
                bv_bc = pp.tile([P, D], F32, name="bv_bc")

                # warm the exp activation table while DMAs stream
                warm = pp.tile([1, 2], F32)
                nc.vector.memset(warm[:], 0.0)
                nc.scalar.activation(warm[:], warm[:], EXP, scale=0.125)

                # PE p-state ramp: ~4us of back-to-back dummy matmuls during
                # the input DMA wait so the first real projections run at full
                # clock instead of 0.65 GHz
                ramp_w = pp.tile([P, 16], F16, name="ramp_w")
                ramp_mv = pp.tile([P, NW], F16, name="ramp_mv")
                nc.vector.memset(ramp_w[:], 0.0)
                nc.vector.memset(ramp_mv[:], 0.0)
                ramp_ps = psp.tile([P, NW], F32, tag="pj", bufs=2, name="ps_pj")
                for _ in range(12):
                    nc.tensor.matmul(
                        ramp_ps[0:16, :], ramp_w[:], ramp_mv[:], start=True, stop=True
                    )

                # ---- input DMA --------------------------------------------
                # Three parallel DMA queues: SP-HWDGE, ACT-HWDGE, Pool-SWDGE.
                # The Pool (gpsimd) queue can CAST f32->f16 in flight, so the
                # bulk of the inputs loads with no staging/rounding step at
                # all; only the three head-critical chunks (xk nw0, xq nw0/1)
                # go as f32 on the two HW queues in parallel, rounded by the
                # otherwise-idle DVE.
                def stage_x(dram, dst, nw, tag, dma, cp):
                    sl = slice(nw * NW, (nw + 1) * NW)
                    st = isp.tile([P, DC, NW], F32, tag=tag, name=f"st_{tag}")
                    dma.dma_start(st[:], dram.rearrange("(dc p) n -> p dc n", p=P)[:, :, sl])
                    cp.tensor_copy(dst[:, :, sl], st[:])

                def cast_x(dram, dst, nw):
                    sl = slice(nw * NW, (nw + 1) * NW)
                    nc.gpsimd.dma_start(
                        dst[:, :, sl], dram.rearrange("(dc p) n -> p dc n", p=P)[:, :, sl]
                    )

                # weights cast f32->f16 in flight on the Pool queue (no
                # staging); the three head-critical x chunks staged f32 on the
                # SP/ACT queues in parallel and rounded by the idle DVE
                # HWDGE-only input staging (SWDGE casting DMAs measure ~23GB/s
                # on HW vs ~75GB/s HWDGE — far too slow for bulk). Transfers
                # are issued in consumption-deadline order, alternating the
                # SP/ACT queues; rounding copies: DVE for the head-critical
                # chunks, Pool for the bulk.
                def stage_w(dram_ap, dst_ap, tag, dma, cp):
                    st = isp.tile([P, DC, D], F32, tag=tag, bufs=1, name=f"st_{tag}")
                    dma.dma_start(st[:], dram_ap)
                    cp.tensor_copy(dst_ap, st[:])

                stage_w(wk_d.rearrange("(dc p) o -> p dc o", p=P), wk16[:], "wk", nc.sync, nc.vector)
                stage_x(xk_d, xk16, 0, "xk", nc.scalar, nc.vector)
                stage_w(wq_d.rearrange("(dc p) o -> p dc o", p=P), wq16[:], "wq", nc.sync, nc.vector)
                stage_x(xq_d, xq16, 0, "xq", nc.scalar, nc.vector)
                stage_x(xq_d, xq16, 1, "xq2", nc.sync, nc.vector)
                nc.sync.dma_start(bq_sb[:], bq_d.rearrange("(c p) -> p c", p=P))
                nc.sync.dma_start(bk_sb[:], bk_d.rearrange("(c p) -> p c", p=P))
                stage_w(wv_d.rearrange("(dc p) o -> p dc o", p=P), wv16[:], "wv", nc.scalar, nc.gpsimd)

                # ---- emission helpers -----------------------------------
                def emit_qkproj(w16, x16, b_sb, dst8, c, nw, aux=None):
                    # projection chunk c (fp8 pair dim), n-window nw -> dst8[:, c, nw]
                    sl = slice(nw * NW, (nw + 1) * NW)
                    ps = psp.tile([P, NW], F32, tag="pj", bufs=2, name="ps_pj")
                    for dc in range(DC):
                        nc.tensor.matmul(
                            ps[:],
                            w16[:, dc, c * P : (c + 1) * P],
                            x16[:, dc, sl],
                            start=(dc == 0),
                            stop=(dc == DC - 1),
                        )
                    nc.vector.tensor_add(
                        out=dst8[:, c, sl],
                        in0=ps[:],
                        in1=b_sb[:, c : c + 1].to_broadcast((P, NW)),
                    )
                    if aux is not None:
                        nc.sync.dma_start(aux[:, c, sl], dst8[96:128, c, sl])

                def emit_vproj(mc):
                    ps = psp.tile([P, NW], F32, tag="pj", bufs=2, name="ps_pj")
                    for dc in range(DC):
                        nc.tensor.matmul(
                            ps[:, 0:D],
                            xv16[:, dc, mc * P : (mc + 1) * P],
                            wv16[:, dc, :],
                            start=(dc == 0),
                            stop=(dc == DC - 1),
                        )
                    nc.vector.tensor_add(
                        out=v16[:, mc, :, 0:HD],
                        in0=ps[:, 0:D].rearrange("p (h e) -> p h e", e=HD),
                        in1=bv_bc[:].rearrange("p (h e) -> p h e", e=HD),
                    )

                def oproj_mm(out_ap, wp, oc, nw, h, start, stop):
                    n0 = wp * WIN + nw * NW
                    nc.tensor.matmul(
                        out_ap,
                        wm_r[:, h, oc * P : (oc + 1) * P],
                        xst[:, h, n0 : n0 + NW],
                        start=start,
                        stop=stop,
                    )

                def oproj_store(in_ap, wp, oc, nw, dma=None, act_add=False):
                    # bias-add reads PSUM: DVE mid-kernel, ACT (idle once the
                    # exps are done) for the tail stores
                    n0 = wp * WIN + nw * NW
                    o_sb = sp.tile([P, NW], F32, tag="ost", name="o_sb")
                    if act_add:
                        nc.scalar.add(o_sb[:], in_ap, bm_sb[:, oc : oc + 1])
                    else:
                        nc.vector.tensor_add(
                            out=o_sb[:],
                            in0=in_ap,
                            in1=bm_sb[:, oc : oc + 1].to_broadcast((P, NW)),
                        )
                    (dma or nc.sync).dma_start(
                        out_d.rearrange("(c p) n -> p c n", p=P)[:, oc, n0 : n0 + NW],
                        o_sb[:],
                    )

                def emit_oproj(wp, oc, nw, dma=None):
                    # out chunk [128 o, 512 n]; accumulate over heads
                    ps = psp.tile([P, NW], F32, tag="pj", bufs=2, name="ps_pj")
                    for h in range(H):
                        oproj_mm(ps[:], wp, oc, nw, h, h == 0, h == H - 1)
                    oproj_store(ps[:], wp, oc, nw, dma=dma)

                def push_oproj_fillers(wp, oc, nw):
                    # per-head matmul fillers so pops stay fine-grained
                    cell = {}

                    def start(oc=oc, nw=nw):
                        cell["ps"] = psp.tile([P, NW], F32, tag="pj", bufs=2, name="ps_pj")
                        oproj_mm(cell["ps"][:], wp, oc, nw, 0, True, False)

                    filler.append((512, start))
                    for hh in range(1, H - 1):
                        filler.append((512, lambda hh=hh: oproj_mm(cell["ps"][:], wp, oc, nw, hh, False, False)))

                    def fin(oc=oc, nw=nw):
                        oproj_mm(cell["ps"][:], wp, oc, nw, H - 1, False, True)
                        oproj_store(cell["ps"][:], wp, oc, nw)

                    filler.append((512, fin))

                # ---- fillers: (estimated PE cycles, closure) ------------
                # deadline order: PV(h0, mc) fires at exp-slot mc+PV_LAG and
                # scores(h0, mc) at slot mc, so vproj(mc) and kproj(nw) must
                # pop (at ~2/slot in the first block) before those slots
                filler = deque()

                def fv(mc):
                    filler.append((512, lambda: emit_vproj(mc)))

                def fk(c, nw):
                    filler.append((1024, lambda: emit_qkproj(wk16, xk16, bk_sb, k8, c, nw, aux=k8b)))

                fv(0); fv(1); fk(0, 1); fk(1, 1); fv(2); fv(3); fv(4); fv(5)
                fk(0, 2); fk(1, 2); fv(6); fv(7); fv(8); fv(9); fk(0, 3); fk(1, 3)
                for mc in range(10, 16):
                    fv(mc)
                for nw in range(2, 4):
                    for c in range(2):
                        filler.append((1024, lambda c=c, nw=nw: emit_qkproj(wq16, xq16, bq_sb, q8, c, nw, aux=q8b)))

                def pop_fillers(budget):
                    while filler and budget > 0:
                        cyc, fn = filler.popleft()
                        fn()
                        budget -= cyc

                # ---- early projections (feed wp0/h0 scores) -------------
                for c in range(2):
                    emit_qkproj(wk16, xk16, bk_sb, k8, c, 0, aux=k8b)
                for nw in range(2):
                    for c in range(2):
                        emit_qkproj(wq16, xq16, bq_sb, q8, c, nw, aux=q8b)

                # bulk loads behind the head-critical transfers: biases, the
                # ones columns of every PV stationary (denominator rows),
                # remaining x chunks, and the f32r out-proj weight
                nc.sync.dma_start(
                    bv_bc[:], bv_d[:].rearrange("(a o) -> a o", a=1).to_broadcast((P, D))
                )
                nc.sync.dma_start(bm_sb[:], bm_d.rearrange("(c p) -> p c", p=P))
                nc.gpsimd.memset(v16[:, :, :, HD : 2 * HD], 1.0)
                stage_x(xk_d, xk16, 1, "xk", nc.scalar, nc.gpsimd)
                stage_x(xv_d, xv16, 0, "xv", nc.sync, nc.gpsimd)
                stage_x(xk_d, xk16, 2, "xk", nc.scalar, nc.gpsimd)
                stage_x(xv_d, xv16, 1, "xv", nc.sync, nc.gpsimd)
                stage_x(xk_d, xk16, 3, "xk", nc.scalar, nc.gpsimd)
                stage_x(xv_d, xv16, 2, "xv", nc.sync, nc.gpsimd)
                stage_x(xv_d, xv16, 3, "xv", nc.scalar, nc.gpsimd)
                stage_x(xq_d, xq16, 2, "xq", nc.sync, nc.gpsimd)
                stage_x(xq_d, xq16, 3, "xq", nc.scalar, nc.gpsimd)
                # wm stays f32r (output-projection precision) — stage + round
                st_wm = isp.tile([HD, H, D], F32, tag="wm", bufs=1, name="st_wm")
                nc.sync.dma_start(st_wm[:], wm_d.rearrange("(h p) o -> p h o", p=HD))
                nc.gpsimd.tensor_copy(wm_r[:], st_wm[:])

                # ---- attention ------------------------------------------
                # PV work queue, carried across blocks: ("pv", fn) entries are
                # rate-limited; ("epi", fn) entries (a block's epilogue, which
                # must be EMITTED after that block's last PV so the dependency
                # tracker orders it correctly) pop for free right after.
                pv_pending = deque()

                def pop_pv(max_pops, lag):
                    pops = 0
                    while pv_pending and pops < max_pops:
                        n_pv = sum(1 for k, _ in pv_pending if k == "pv")
                        if n_pv <= lag and pv_pending[0][0] == "pv":
                            break
                        kind, fn = pv_pending.popleft()
                        fn()
                        if kind == "pv":
                            pops += 1

                def emit_epilogue(h, wp, x_ps, halves=1):
                    # Engines cannot shift data across partitions, so the
                    # denominator (rows 64:128 of the accumulator) reaches
                    # partitions 0:63 via a DRAM bounce, as a 64-partition
                    # broadcast. One DVE copy first frees the single-buffered
                    # PSUM x within the PV pop pause.
                    xu = rp.tile([P, WIN], F32, tag="xu", name="xu")
                    nc.vector.tensor_copy(xu[:], x_ps[:])
                    den_dr = dsp.tile([1, WIN], F32, tag="den", name="den_dr")
                    nc.gpsimd.dma_start(den_dr[:], xu[HD : HD + 1, :])
                    bc = rp.tile([HD, WIN], F32, tag="bc", name="bc")
                    nc.gpsimd.dma_start(bc[:], den_dr[:].to_broadcast((HD, WIN)))
                    hw = WIN // halves
                    for q in range(halves):
                        sl = slice(q * hw, (q + 1) * hw)
                        nc.vector.reciprocal_approx_fast(
                            out=bc[:, sl], in_=bc[:, sl]
                        )
                        nc.vector.tensor_mul(
                            out=xst[:, h, wp * WIN + q * hw : wp * WIN + (q + 1) * hw],
                            in0=xu[0:HD, sl],
                            in1=bc[:, sl],
                        )
                        yield q

                for wp in range(N // WIN):
                    for h in range(H):
                        hb = h * 32
                        last = wp == N // WIN - 1 and h == H - 1
                        o_partials = {}
                        # x accumulator allocated lazily at the first PV pop so
                        # its WAR dep against the previous block's (deferred)
                        # epilogue copy is seen by the dependency tracker
                        cell = {}

                        def get_x(cell=cell):
                            if "x" not in cell:
                                cell["x"] = psp.tile([P, WIN], F32, tag="x", bufs=1, name="x_ps")
                            return cell["x"]

                        def emit_pv(mc, e16, get_x=get_x, h=h):
                            x_ps = get_x()
                            for j in range(WIN // NW):
                                nc.tensor.matmul(
                                    x_ps[:, j * NW : (j + 1) * NW],
                                    v16[:, mc, h, :],
                                    e16[:, j * NW : (j + 1) * NW],
                                    start=(mc == 0),
                                    stop=(mc == MC - 1),
                                )

                        if h < 3:
                            k_src = k8[hb : hb + 32, :, :]
                            q_src = q8[hb : hb + 32, :, :]
                        else:
                            k_src = k8b[:, :, :]
                            q_src = q8b[:, :, :]
                        for mc in range(MC):
                            s_ps = psp.tile([P, WIN], F32, tag="s", bufs=2, name="s_ps")
                            for j in range(WIN // NW):
                                n0 = wp * WIN + j * NW
                                nc.tensor.matmul(
                                    s_ps[:, j * NW : (j + 1) * NW],
                                    k_src[:, :, mc * P : (mc + 1) * P],
                                    q_src[:, :, n0 : n0 + NW],
                                    start=True,
                                    stop=True,
                                    perf_mode=DR,
                                )
                            e16 = ep.tile([P, WIN], F16, tag="e", name="e16")
                            nc.scalar.activation(e16[:], s_ps[:], EXP, scale=0.125)
                            pv_pending.append(
                                ("pv", lambda mc=mc, e16=e16, emit_pv=emit_pv: emit_pv(mc, e16))
                            )
                            pop_fillers(1600 if wp == 0 and h == 0 else 900)
                            # PV pops pause for the first 4 slots of each block
                            # so the previous block's epilogue can free the
                            # single-buffered PSUM accumulator
                            if mc >= 4 or (wp == 0 and h == 0):
                                pop_pv(2, PV_LAG)
                            if last and mc in (6, 9):
                                # pre-accumulate heads 0..2 of the oc=0 output
                                # chunks so only h3's matmul trails the final
                                # epilogue
                                nwp = 0 if mc == 6 else 1
                                ps = psp.tile([P, NW], F32, tag="pj", bufs=2, name="ps_pj")
                                for hh in range(H - 1):
                                    oproj_mm(ps[:], wp, 0, nwp, hh, hh == 0, False)
                                o_partials[(0, nwp)] = ps
                        if last:
                            pop_pv(1 << 30, 0)
                            # tail: copy x out of PSUM per half, then reuse the
                            # freed x banks to pre-accumulate heads 0..2 of the
                            # oc=1 chunks too, so only h3's matmuls + bias +
                            # store trail the per-half epilogue
                            # oc=1 partials go into the score-tile rotation,
                            # free as soon as the second-to-last exp finishes —
                            # they overlap the x copies instead of waiting on
                            # them
                            x2 = psp.tile([P, WIN], F32, tag="s", bufs=2, name="x2")
                            for nw in range(WIN // NW):
                                sl = slice(nw * NW, (nw + 1) * NW)
                                for hh in range(H - 1):
                                    oproj_mm(x2[:, sl], wp, 1, nw, hh, hh == 0, False)
                                o_partials[(1, nw)] = (x2, sl)
                            xx = get_x()
                            dmas = [nc.sync, nc.scalar]
                            for nw in emit_epilogue(h, wp, xx, halves=WIN // NW):
                                ps = o_partials.pop((0, nw))
                                oproj_mm(ps[:], wp, 0, nw, H - 1, False, True)
                                oproj_store(ps[:], wp, 0, nw, dma=dmas[0], act_add=True)
                                x2t, x2sl = o_partials.pop((1, nw))
                                oproj_mm(x2t[:, x2sl], wp, 1, nw, H - 1, False, True)
                                oproj_store(x2t[:, x2sl], wp, 1, nw, dma=dmas[1], act_add=True)
                        else:

                            def epi(h=h, wp=wp, get_x=get_x):
                                for _ in emit_epilogue(h, wp, get_x()):
                                    pass
                                if h == H - 1:
                                    for oc in range(DC):
                                        for nw in range(WIN // NW):
                                            push_oproj_fillers(wp, oc, nw)

                            pv_pending.append(("epi", epi))
                pop_fillers(1 << 30)  # drain anything left

    nc.finalize()
    return nc


_NC_CACHE = None


def _get_nc():
    global _NC_CACHE
    if _NC_CACHE is None:
        _NC_CACHE = build_nc()
    return _NC_CACHE


# fp8 DoubleRow q/k layout: permuted column j = c*128 + h*32 + dl holds
# original output channel o = d*H + h with d = 32*c + dl  (heads in
# 32-partition bands, d split across the fp8 pair dim c).
_QK_PERM = np.empty(D, np.int64)
for _j in range(D):
    _c, _r = divmod(_j, P)
    _h, _dl = divmod(_r, 32)
    _QK_PERM[_j] = (32 * _c + _dl) * H + _h
# v/wm: col j = h*64 + dd maps to o = dd*H + h (head-contiguous)
_V_PERM = np.empty(D, np.int64)
for _j in range(D):
    _h, _dd = divmod(_j, HD)
    _V_PERM[_j] = _dd * H + _h


def make_in_maps(**inputs: np.ndarray) -> list:
    query = np.ascontiguousarray(np.asarray(inputs["query"], np.float32))
    key = np.ascontiguousarray(np.asarray(inputs["key"], np.float32))
    value = np.ascontiguousarray(np.asarray(inputs["value"], np.float32))
    wq = np.ascontiguousarray(np.asarray(inputs["Wq"], np.float32)[:, _QK_PERM])
    wk = np.ascontiguousarray(np.asarray(inputs["Wk"], np.float32)[:, _QK_PERM])
    wv = np.ascontiguousarray(np.asarray(inputs["Wv"], np.float32)[:, _V_PERM])
    wm = np.ascontiguousarray(np.asarray(inputs["Wm"], np.float32)[_V_PERM, :])
    bq = np.ascontiguousarray(np.asarray(inputs["bq"], np.float32)[_QK_PERM])
    bk = np.ascontiguousarray(np.asarray(inputs["bk"], np.float32)[_QK_PERM])
    bv = np.ascontiguousarray(np.asarray(inputs["bv"], np.float32)[_V_PERM])
    bm = np.ascontiguousarray(np.asarray(inputs["bm"], np.float32))

    return [
        {
            "query": query[b],
            "key": key[b],
            "value": value[b],
            "wq": wq,
            "wk": wk,
            "wv": wv,
            "wm": wm,
            "bq": bq,
            "bk": bk,
            "bv": bv,
            "bm": bm,
        }
        for b in range(B)
    ]


def kernel(**inputs: np.ndarray) -> np.ndarray:
    nc = _get_nc()
    in_maps = make_in_maps(**inputs)
    res = run_bass_kernel_spmd(nc, in_maps, core_ids=list(range(B)))
    global _LAST_RESULT
    _LAST_RESULT = res
    return np.stack([r["out"] for r in res.results], axis=0)


_LAST_RESULT = None


# revision 62
# speedup vs baseline: 1.1269x; 1.0972x over previous
"""Multi-head attention Trainium2 Bass kernel.

Problem: nn_MultiHeadAttention (B=8, D=256, N=2048, H=4, head_dim=64), fp32.

Sharding: data-parallel over batch — each of the 8 NeuronCores handles one
batch element end to end (no communication needed).

Per-core algorithm:
  - Q/K projections and the score matmuls run in bf16: score noise passes
    through exp() as a tiny multiplicative perturbation of the softmax
    weights (~2e-4), which the value-averaging does not amplify.
  - The V path (V^T projection, PV matmul, output projection) runs in
    float32r (~full PE speed for free-dim >= 256, much better precision
    than bf16) because value-path noise lands on the output directly.
  - Scores are computed transposed, S^T[m, n] = sum_d k[d,m] q[d,n], so no
    operand ever needs a transpose; exp(S^T/8) runs on the scalar engine
    straight out of PSUM (scale fused into the ACTIVATE). Max-subtraction
    is skipped — scores are O(1) here, exp cannot overflow.
  - A ones-column appended to each head's V^T makes the PV matmul emit the
    softmax denominator as an extra PSUM row (row 64); lhsT is padded to 66
    columns because fp32r requires an even stationary free size.
  - fp32r operands must be produced (rounded) by a compute engine, so
    DMA-loaded tensors pass through one DVE copy into bf16/fp32r tiles.
"""

import numpy as np

import concourse.bass as bass
import concourse.bacc as bacc
import concourse.mybir as mybir
import concourse.tile as tile
from concourse.bass_utils import run_bass_kernel_spmd

F32 = mybir.dt.float32
F32R = mybir.dt.float32r
BF16 = mybir.dt.bfloat16
F16 = mybir.dt.float16
EXP = mybir.ActivationFunctionType.Exp

B, D, N, H = 8, 256, 2048, 4
HD = D // H  # 64
P = 128
DC = D // P  # 2 d-chunks
MC = N // P  # 16 m-chunks
NW = 512     # matmul free-dim chunk
WIN = 1024   # exp window (psum scores tile width)
VW = HD + 2  # PV stationary width: 64 v-cols + ones + zero pad (must be even)


def build_nc(debug_taps: bool = False, reps: int = 1, probe: str = '') -> bass.Bass:
    nc = bacc.Bacc()
    assert not debug_taps, "debug taps removed in window-outer version"

    xq_d = nc.declare_dram_parameter("query", [D, N], F32, isOutput=False)
    xk_d = nc.declare_dram_parameter("key", [D, N], F32, isOutput=False)
    xv_d = nc.declare_dram_parameter("value", [D, N], F32, isOutput=False)
    wq_d = nc.declare_dram_parameter("wq", [D, D], F32, isOutput=False)
    wk_d = nc.declare_dram_parameter("wk", [D, D], F32, isOutput=False)
    wv_d = nc.declare_dram_parameter("wv", [D, D], F32, isOutput=False)
    wm_d = nc.declare_dram_parameter("wm", [D, D], F32, isOutput=False)
    bq_d = nc.declare_dram_parameter("bq", [D], F32, isOutput=False)
    bk_d = nc.declare_dram_parameter("bk", [D], F32, isOutput=False)
    bv_d = nc.declare_dram_parameter("bv", [D], F32, isOutput=False)
    bm_d = nc.declare_dram_parameter("bm", [D], F32, isOutput=False)
    out_d = nc.declare_dram_parameter("out", [D, N], F32, isOutput=True)

    with tile.TileContext(nc) as tc:
        for _rep in range(reps):
            with (
                tc.tile_pool(name="persist", bufs=1) as pp,
                tc.tile_pool(name="stage", bufs=2) as sp,
            ):
                isp = tc.alloc_tile_pool(name="instage", bufs=1)
                # ---- load + round inputs ----------------------------------------
                # fp32r/bf16 matmul operands must be rounded by a compute engine,
                # so every DMA-loaded tensor passes through one DVE copy. Each
                # input gets its own stage tile so the input DMAs carry no sync
                # waits (the HWDGE DMA pseudo-instruction has very few wait slots).
                def load_round(dram_ap, shape, dtype, name, split=1):
                    st = isp.tile(shape, F32, tag=f"st_{name}", name=f"st_{name}")
                    t = pp.tile(shape, dtype, name=name)
                    # split along dim 1 so consumers of the first chunk start
                    # before the whole tensor is staged + rounded
                    step = shape[1] // split
                    for s0 in range(0, shape[1], step):
                        sl = slice(s0, s0 + step)
                        nc.sync.dma_start(st[:, sl], dram_ap[:, sl])
                        nc.vector.tensor_copy(t[:, sl], st[:, sl])
                    return t

                wq_b = load_round(
                    wq_d.rearrange("(dc p) o -> p dc o", p=P), [P, DC, D], F16, "wq_b"
                )
                xq_b = load_round(
                    xq_d.rearrange("(dc p) n -> p dc n", p=P), [P, DC, N], F16, "xq_b", split=DC
                )
                wk_b = load_round(
                    wk_d.rearrange("(dc p) o -> p dc o", p=P), [P, DC, D], F16, "wk_b"
                )
                xk_b = load_round(
                    xk_d.rearrange("(dc p) n -> p dc n", p=P), [P, DC, N], F16, "xk_b", split=DC
                )
                wv_r = load_round(
                    wv_d.rearrange("(dc p) o -> p dc o", p=P), [P, DC, D], F32R, "wv_r"
                )
                xv_r = load_round(
                    xv_d.rearrange("(dc p) n -> p dc n", p=P), [P, DC, N], F32R, "xv_r", split=DC
                )
                wm_r = load_round(
                    wm_d.rearrange("(h p) o -> p h o", p=HD), [HD, H, D], F32R, "wm_r"
                )

                bv_bc = pp.tile([P, D], F32)
                nc.sync.dma_start(
                    bv_bc[:], bv_d[:].rearrange("(a o) -> a o", a=1).to_broadcast((P, D))
                )
                bq_sb = pp.tile([P, DC], F32)
                nc.sync.dma_start(bq_sb[:], bq_d.rearrange("(c p) -> p c", p=P))
                bk_sb = pp.tile([P, DC], F32)
                nc.sync.dma_start(bk_sb[:], bk_d.rearrange("(c p) -> p c", p=P))
                bm_sb = pp.tile([P, DC], F32)
                nc.sync.dma_start(bm_sb[:], bm_d.rearrange("(c p) -> p c", p=P))

                # warm the exp activation-table while input DMAs stream: the
                # ~2.7us ACT_TABLE_LOAD fires before the first Exp in ACT
                # program order, so a dummy exp here pulls it off the
                # attention critical path (ACT is otherwise idle at start).
                warm = pp.tile([1, 2], F32)
                nc.vector.memset(warm[:], 0.0)
                nc.scalar.activation(warm[:], warm[:], EXP, scale=0.125)

                # ---- persistent compute tiles -----------------------------------
                q_sb = pp.tile([P, DC, N], F16)
                k_sb = pp.tile([P, DC, N], F16)
                vT_sb = pp.tile([P, MC, H, VW], F32R)
                # memset can't write float32r — round a small f32 [1, 0] pair in
                ones2 = pp.tile([P, 2], F32)
                nc.vector.memset(ones2[:, 0:1], 1.0)
                nc.vector.memset(ones2[:, 1:2], 0.0)
                nc.vector.tensor_copy(
                    vT_sb[:, :, :, HD : HD + 2],
                    ones2.unsqueeze(1).unsqueeze(1).to_broadcast((P, MC, H, 2)),
                )
                xst_sb = pp.tile([HD, H, N], F32R)  # normalized per-head attn out

                isp.release()  # staging range reused by the attention pools below

                # ---- projections -------------------------------------------------
                # q/k chunk 0 first so head-0 attention can start early, then
                # v^T (PV consumes it m-chunk by m-chunk), then q/k chunk 1.
                with tc.tile_pool(name="psum_proj", bufs=2, space="PSUM") as pjp:

                    def emit_qk(w_sb, x_sb, b_sb, dst, oc):
                        for nw in range(N // NW):
                            ps_p = pjp.tile([P, NW], F32, tag="pqk", name="ps_p")
                            for dc in range(DC):
                                nc.tensor.matmul(
                                    ps_p[:],
                                    w_sb[:, dc, oc * P : (oc + 1) * P],
                                    x_sb[:, dc, nw * NW : (nw + 1) * NW],
                                    start=(dc == 0),
                                    stop=(dc == DC - 1),
                                )
                            nc.vector.tensor_add(
                                out=dst[:, oc, nw * NW : (nw + 1) * NW],
                                in0=ps_p[:],
                                in1=b_sb[:, oc : oc + 1].to_broadcast((P, NW)),
                            )

                    emit_qk(wq_b, xq_b, bq_sb, q_sb, 0)
                    emit_qk(wk_b, xk_b, bk_sb, k_sb, 0)

                    # v^T : (n-chunk 128, o 256), accumulated over d-chunks
                    for mc in range(MC):
                        ps_v = pjp.tile([P, D], F32, tag="pv")
                        for dc in range(DC):
                            nc.tensor.matmul(
                                ps_v[:],
                                xv_r[:, dc, mc * P : (mc + 1) * P],
                                wv_r[:, dc, :],
                                start=(dc == 0),
                                stop=(dc == DC - 1),
                            )
                        nc.vector.tensor_add(
                            out=vT_sb[:, mc, :, 0:HD],
                            in0=ps_v[:].rearrange("p (h e) -> p h e", e=HD),
                            in1=bv_bc[:].rearrange("p (h e) -> p h e", e=HD),
                        )

                    emit_qk(wq_b, xq_b, bq_sb, q_sb, 1)
                    emit_qk(wk_b, xk_b, bk_sb, k_sb, 1)


                # ---- attention ---------------------------------------------------
                with (
                    tc.tile_pool(name="psum_att", bufs=1, space="PSUM") as pa,
                    tc.tile_pool(name="exp_pool", bufs=6) as ep,
                    tc.tile_pool(name="rbc_pool", bufs=3) as rp,
                    tc.tile_pool(name="dram_scr", bufs=4, space="DRAM") as dsp,
                ):
                    # Head-pair processing: the two heads of each q/k chunk
                    # live at partition bases 0 and 64, so their score matmuls
                    # target different PE row groups and overlap in the array
                    # (weight loads included). Window-outer keeps two (66, WIN)
                    # x accumulators + double-buffered score tiles in 8 banks.
                    for hc in range(DC):
                        for w in range(N // WIN):
                            x_ps = [
                                pa.tile([VW, WIN], F32, tag=f"x{i}", bufs=1, name="x_ps")
                                for i in range(2)
                            ]

                            def emit_pv(mc, e_pair):
                                for i in range(2):
                                    for j in range(WIN // NW):
                                        nc.tensor.matmul(
                                            x_ps[i][:, j * NW : (j + 1) * NW],
                                            vT_sb[:, mc, hc * 2 + i, :],
                                            e_pair[i][:, j * NW : (j + 1) * NW],
                                            start=(mc == 0),
                                            stop=(mc == MC - 1),
                                        )

                            prev = None
                            for mc in range(MC):
                                e_pair = []
                                for i in range(2):
                                    hb = i * HD
                                    s_ps = pa.tile(
                                        [P, WIN], F32, tag="s", bufs=2, name="s_ps"
                                    )
                                    for j in range(WIN // NW):
                                        n0 = w * WIN + j * NW
                                        nc.tensor.matmul(
                                            s_ps[:, j * NW : (j + 1) * NW],
                                            k_sb[hb : hb + HD, hc, mc * P : (mc + 1) * P],
                                            q_sb[hb : hb + HD, hc, n0 : n0 + NW],
                                            start=True,
                                            stop=True,
                                        )
                                    e_sb = ep.tile([P, WIN], F32R, tag="e", name="e_sb")
                                    nc.scalar.activation(
                                        e_sb[:], s_ps[:], EXP, scale=0.125
                                    )
                                    e_pair.append(e_sb)
                                if prev is not None:
                                    emit_pv(*prev)
                                prev = (mc, e_pair)
                            emit_pv(*prev)

                            # epilogue per head: one (65, WIN) DVE copy moves
                            # x_unnorm + denominator out of PSUM; reciprocal is
                            # broadcast to partitions 0..63 via a DRAM bounce.
                            n0 = w * WIN
                            for i in range(2):
                                h = hc * 2 + i
                                xu = rp.tile(
                                    [HD + 1, WIN], F32, tag="xu", bufs=3, name="xu"
                                )
                                nc.vector.tensor_copy(xu[:], x_ps[i][0 : HD + 1, :])
                                rden_dr = dsp.tile(
                                    [1, WIN], F32, tag="dden", name="rden_dr"
                                )
                                nc.gpsimd.dma_start(rden_dr[:], xu[HD : HD + 1, :])
                                rden_bc = rp.tile(
                                    [HD, WIN], F32, tag="rbc", name="rden_bc"
                                )
                                nc.gpsimd.dma_start(
                                    rden_bc[:], rden_dr[:].to_broadcast((HD, WIN))
                                )
                                nc.vector.reciprocal_approx_fast(
                                    out=rden_bc[:], in_=rden_bc[:]
                                )
                                nc.vector.tensor_mul(
                                    out=xst_sb[:, h, n0 : n0 + WIN],
                                    in0=xu[0:HD, :],
                                    in1=rden_bc[:],
                                )

                # ---- output projection ------------------------------------------
                with tc.tile_pool(name="psum_out", bufs=4, space="PSUM") as po:
                    for oc in range(DC):
                        # 4 concurrent accumulators so each wm slice is loaded
                        # once and streams all four n-chunks (h loop outer)
                        ps_os = [
                            po.tile([P, NW], F32, tag="po", name="ps_o")
                            for _ in range(N // NW)
                        ]
                        for h in range(H):
                            for nw in range(N // NW):
                                nc.tensor.matmul(
                                    ps_os[nw][:],
                                    wm_r[:, h, oc * P : (oc + 1) * P],
                                    xst_sb[:, h, nw * NW : (nw + 1) * NW],
                                    start=(h == 0),
                                    stop=(h == H - 1),
                                )
                        for nw in range(N // NW):
                            o_sb = sp.tile([P, NW], F32, tag="ostage", name="o_sb")
                            nc.vector.tensor_add(
                                out=o_sb[:],
                                in0=ps_os[nw][:],
                                in1=bm_sb[:, oc : oc + 1].to_broadcast((P, NW)),
                            )
                            nc.sync.dma_start(
                                out_d.rearrange("(c p) n -> p c n", p=P)[
                                    :, oc, nw * NW : (nw + 1) * NW
                                ],
                                o_sb[:],
                            )

    nc.finalize()
    return nc


_NC_CACHE = None


def _get_nc():
    global _NC_CACHE
    if _NC_CACHE is None:
        _NC_CACHE = build_nc()
    return _NC_CACHE


# column j of the permuted Wq/Wk maps to original output channel o = hd*H + h
# with j = (h // 2) * 128 + (h % 2) * 64 + hd  (head-contiguous, chunk-split)
_QK_PERM = np.empty(D, np.int64)
for _j in range(D):
    _c, _rr = divmod(_j, P)
    _h2, _hd = divmod(_rr, HD)
    _QK_PERM[_j] = _hd * H + (_c * 2 + _h2)
# column j of the permuted Wv maps to o = hd*H + h with j = h*64 + hd
_V_PERM = np.empty(D, np.int64)
for _j in range(D):
    _h, _hd = divmod(_j, HD)
    _V_PERM[_j] = _hd * H + _h


def make_in_maps(**inputs: np.ndarray) -> list:
    query = np.ascontiguousarray(np.asarray(inputs["query"], np.float32))
    key = np.ascontiguousarray(np.asarray(inputs["key"], np.float32))
    value = np.ascontiguousarray(np.asarray(inputs["value"], np.float32))
    wq = np.ascontiguousarray(np.asarray(inputs["Wq"], np.float32)[:, _QK_PERM])
    wk = np.ascontiguousarray(np.asarray(inputs["Wk"], np.float32)[:, _QK_PERM])
    wv = np.ascontiguousarray(np.asarray(inputs["Wv"], np.float32)[:, _V_PERM])
    wm = np.ascontiguousarray(np.asarray(inputs["Wm"], np.float32)[_V_PERM, :])
    bq = np.ascontiguousarray(np.asarray(inputs["bq"], np.float32)[_QK_PERM])
    bk = np.ascontiguousarray(np.asarray(inputs["bk"], np.float32)[_QK_PERM])
    bv = np.ascontiguousarray(np.asarray(inputs["bv"], np.float32)[_V_PERM])
    bm = np.ascontiguousarray(np.asarray(inputs["bm"], np.float32))

    return [
        {
            "query": query[b],
            "key": key[b],
            "value": value[b],
            "wq": wq,
            "wk": wk,
            "wv": wv,
            "wm": wm,
            "bq": bq,
            "bk": bk,
            "bv": bv,
            "bm": bm,
        }
        for b in range(B)
    ]


def kernel(**inputs: np.ndarray) -> np.ndarray:
    nc = _get_nc()
    in_maps = make_in_maps(**inputs)
    res = run_bass_kernel_spmd(nc, in_maps, core_ids=list(range(B)))
    global _LAST_RESULT
    _LAST_RESULT = res
    return np.stack([r["out"] for r in res.results], axis=0)


_LAST_RESULT = None

